# revision 1
# baseline (speedup 1.0000x reference)
"""CFConv (gnn message passing) Trainium2 kernel.

Math (per batch b):
    f1 = ssp(r @ W1 + b1)            ssp(x) = softplus(x) - log2
    f2 = ssp(f1 @ W2 + b2)
    out[i, d] = sum_j x[j, d] * f2[i, j, d]

Sharding: data-parallel over batch B=8 across the 8 cores (one batch each).

Per-core device pipeline (all in "transposed" orientation, features on
partitions, (i,j)-rows on the free dim):
  1. r rows are viewed as pairs [(i,j2), 128] in bf16 and loaded with the
     DMA xbar transpose -> SBUF tile [128, pairs]; partitions 0:64 hold the
     rbf features of even j, 64:128 of odd j.
  2. mm1: two K=64 row-tiled matmuls against a stacked [W1;W1] stationary
     (they run concurrently in different PE row groups).
  3. act1: softplus as Ln(Exp(z1 + b1) + 1) -- two ACT passes (no native
     softplus table on this stack; Exp and Ln share one ACT table set, and
     the table chooser is pinned to it to avoid per-op table reloads).
     Exp goes PSUM -> SBUF f32 per group; Ln runs once per 4-group chunk
     (FD=8192) to amortize the ~0.5 us per-op ACT bubble.
     The "- log2" shift is folded into layer 2's bias (b2' below).
  4. mm2: K=128 matmuls against W2.
  5. act2: same two-pass softplus with bias b2' = b2 - log2 * sum_d W2[d,:].
  6. Final contraction over j on DVE: prod = a2 * xT (xT broadcast over i),
     3D tensor_reduce over the inner j axis, even+odd add, then a
     per-partition add of the "- log2 * sum_j x[j,d]" correction.
     Output stays [d, i] on device; the host transposes back.

Emission is software-pipelined: layer-2 work of chunk c-1 interleaves with
layer-1 work of chunk c at group granularity, so the ACT engine (the
bottleneck at ~1 elem/lane/cycle x 4 passes) never head-of-line blocks on
matmuls.  Measured ~250 us per core on hardware (all 8 cores run in
parallel, one batch each).
"""

import numpy as np
import ml_dtypes

import concourse.bass as bass
import concourse.tile as tile
from concourse import bacc, mybir
from concourse.bass_utils import run_bass_kernel_spmd

LOG2 = float(np.log(2.0))

B, N, D, RBF = 8, 256, 128, 64
PAIRS = N * N // 2            # 32768 row-pairs per batch
CHUNK_PAIRS = 4096            # pairs per DMA-transpose chunk (1 MiB)
GROUP_PAIRS = 1024            # pairs per PSUM group (8 query nodes i)
SUB = 512                     # pairs per matmul (one PSUM bank)
I_PER_GROUP = GROUP_PAIRS // (N // 2)   # 8
H = CHUNK_PAIRS // GROUP_PAIRS          # groups per chunk tile (4)
N_CORES = 8

BF16 = mybir.dt.bfloat16
F32 = mybir.dt.float32


def _build_program(reps: int = 1):
    # Restrict the ACT-table chooser to the one set holding BOTH Exp and Ln;
    # otherwise it alternates between per-function sets and pays a ~2.7us
    # table load on every activation.
    import concourse.bacc as _bacc_mod
    from concourse.hw_specs import get_activation_tables as _gat
    _orig = _gat("gen3")
    _both = mybir.ActivationFunctionType.Exp, mybir.ActivationFunctionType.Ln
    _patched = {
        name: (funcs if name == "natural_log_exp_and_others"
               else type(funcs)(f for f in funcs if f not in _both))
        for name, funcs in _orig.items()
    }
    _bacc_mod.get_activation_tables = lambda arch: _patched

    nc = bacc.Bacc("TRN2", target_bir_lowering=False, debug=False,
                   num_devices=N_CORES)

    rp = nc.dram_tensor("rp", [PAIRS, 2 * RBF], BF16, kind="ExternalInput").ap()
    xte = nc.dram_tensor("xte", [D, N // 2], BF16, kind="ExternalInput").ap()
    xto = nc.dram_tensor("xto", [D, N // 2], BF16, kind="ExternalInput").ap()
    corr = nc.dram_tensor("corr", [D, 1], F32, kind="ExternalInput").ap()
    w1s = nc.dram_tensor("w1s", [2 * RBF, D], BF16, kind="ExternalInput").ap()
    w2 = nc.dram_tensor("w2", [D, D], BF16, kind="ExternalInput").ap()
    b1 = nc.dram_tensor("b1c", [D, 1], F32, kind="ExternalInput").ap()
    b2p = nc.dram_tensor("b2p", [D, 1], F32, kind="ExternalInput").ap()
    outT = nc.dram_tensor("outT", [D, N], F32, kind="ExternalOutput").ap()

    f_exp = mybir.ActivationFunctionType.Exp
    f_ln = mybir.ActivationFunctionType.Ln
    mult = mybir.AluOpType.mult
    add = mybir.AluOpType.add

    with tile.TileContext(nc) as tc:
        with (
            tc.tile_pool(name="const", bufs=1) as const,
            tc.tile_pool(name="rt", bufs=3) as rt_pool,
            tc.tile_pool(name="e1", bufs=1) as e1_pool,
            tc.tile_pool(name="e2", bufs=1) as e2_pool,
            tc.tile_pool(name="a1", bufs=2) as a1_pool,
            tc.tile_pool(name="a2", bufs=2) as a2_pool,
            tc.tile_pool(name="prod", bufs=2) as prod_pool,
            tc.tile_pool(name="acc", bufs=2) as acc_pool,
            tc.tile_pool(name="osb", bufs=1) as out_pool,
            tc.tile_pool(name="f1", bufs=1, space="PSUM") as f1_pool,
            tc.tile_pool(name="f2", bufs=1, space="PSUM") as f2_pool,
        ):
            w1s_t = const.tile([2 * RBF, D], BF16, tag="w1s")
            w2_t = const.tile([D, D], BF16, tag="w2")
            xte_t = const.tile([D, N // 2], BF16, tag="xte")
            xto_t = const.tile([D, N // 2], BF16, tag="xto")
            b1_t = const.tile([D, 1], F32, tag="b1")
            b2p_t = const.tile([D, 1], F32, tag="b2p")
            corr_t = const.tile([D, 1], F32, tag="corr")
            nc.sync.dma_start(w1s_t[:], w1s[:])
            nc.sync.dma_start(w2_t[:], w2[:])
            nc.sync.dma_start(xte_t[:], xte[:])
            nc.sync.dma_start(xto_t[:], xto[:])
            nc.sync.dma_start(b1_t[:], b1[:])
            nc.sync.dma_start(b2p_t[:], b2p[:])
            nc.sync.dma_start(corr_t[:], corr[:])

            out_sb = out_pool.tile([D, N], F32, tag="osb")

            # Tiny warmup activation right after the const loads: hoists the
            # ~2.7us ACT table load to t~0 where it overlaps the first DMA
            # instead of sitting in front of the first real Exp.
            warm = acc_pool.tile([D, 1], F32, tag="warm")
            nc.scalar.activation(warm[:], b1_t[:],
                                 mybir.ActivationFunctionType.Exp, bias=0.0)

            jw = N // 2
            G2 = 2 * GROUP_PAIRS          # cols per group (2048)
            PW = H * G2                   # cols per chunk tile (8192)
            I_PAIR = H * I_PER_GROUP      # 32 query nodes per chunk tile
            xe4 = xte_t[:, None, None, :].broadcast_to([D, H, I_PER_GROUP, jw])
            xo4 = xto_t[:, None, None, :].broadcast_to([D, H, I_PER_GROUP, jw])

            def stage1_half(rt, e1w, h):
                """mm1 + Exp for one group (half pair)."""
                g0 = h * GROUP_PAIRS
                # f1 layout: [even 0:GROUP | odd GROUP:2*GROUP]
                f1 = f1_pool.tile([D, G2], F32, tag="f1")
                for s in range(GROUP_PAIRS // SUB):
                    cs = g0 + s * SUB
                    nc.tensor.matmul(
                        f1[:, s * SUB:(s + 1) * SUB],
                        w1s_t[0:RBF, :],
                        rt[0:RBF, cs:cs + SUB],
                    )
                for s in range(GROUP_PAIRS // SUB):
                    cs = g0 + s * SUB
                    nc.tensor.matmul(
                        f1[:, GROUP_PAIRS + s * SUB:
                            GROUP_PAIRS + (s + 1) * SUB],
                        w1s_t[RBF:2 * RBF, :],
                        rt[RBF:2 * RBF, cs:cs + SUB],
                    )
                # softplus(z1 + b1) = Ln(Exp(z1 + b1) + 1)
                nc.scalar.activation(
                    e1w[:, h * G2:(h + 1) * G2], f1[:], f_exp, bias=b1_t[:])

            def stage2_half(a1w, e2w, h, pool=None, ptag="f2"):
                """mm2 + Exp for one group of the previous pair."""
                f2 = (pool or f2_pool).tile([D, G2], F32, tag=ptag)
                for s in range(G2 // SUB):
                    nc.tensor.matmul(
                        f2[:, s * SUB:(s + 1) * SUB],
                        w2_t[:],
                        a1w[:, h * G2 + s * SUB:h * G2 + (s + 1) * SUB],
                    )
                nc.scalar.activation(
                    e2w[:, h * G2:(h + 1) * G2], f2[:], f_exp, bias=b2p_t[:])

            def stage2_tail(e2w, i0):
                """wide Ln of layer 2 + weighted j-reduction (one pair)."""
                a2w = a2_pool.tile([D, PW], BF16, tag="a2")
                nc.scalar.activation(a2w[:], e2w[:], f_ln, bias=1.0)

                # prod = a2 * xT (x broadcast over h and i); reduce innermost
                # j2; out[:, i] = even + odd + corr.
                # a2w cols = [h: [even 1024 | odd 1024]] * H
                prod = prod_pool.tile([D, PW], BF16, tag="prod")
                p4 = prod[:].rearrange(
                    "p (h par k j) -> p h par k j", h=H, par=2, j=jw)
                a4 = a2w[:].rearrange(
                    "p (h par k j) -> p h par k j", h=H, par=2, j=jw)
                nc.vector.tensor_tensor(
                    p4[:, :, 0, :, :], a4[:, :, 0, :, :], xe4, mult)
                nc.vector.tensor_tensor(
                    p4[:, :, 1, :, :], a4[:, :, 1, :, :], xo4, mult)
                # sums index m = h*16 + par*8 + k
                sums = acc_pool.tile([D, 2 * I_PAIR], F32, tag="sums")
                nc.vector.tensor_reduce(
                    sums[:],
                    prod[:].rearrange("p (m j) -> p m j", j=jw),
                    axis=mybir.AxisListType.X,
                    op=add,
                )
                tmp = acc_pool.tile([D, I_PAIR], F32, tag="tmp")
                s4 = sums[:].rearrange(
                    "p (h par k) -> p h par k", h=H, par=2)
                nc.vector.tensor_add(
                    tmp[:].rearrange("p (h k) -> p h k", h=H),
                    s4[:, :, 0, :], s4[:, :, 1, :])
                nc.vector.tensor_scalar_add(
                    out_sb[:, i0:i0 + I_PAIR], tmp[:], corr_t[:])

            # Software-pipelined emission interleaving halves of pair p's
            # layer 1 with halves of pair p-1's layer 2, so every ACT op has
            # a PE window in front of it and ACT never head-of-line blocks.
            def body():
                pending = None  # (a1w, i0) of the previous pair
                for c in range(PAIRS // CHUNK_PAIRS):
                    rt = rt_pool.tile([2 * RBF, CHUNK_PAIRS], BF16, tag="rt")
                    if c == 0:
                        # Quarter the first transpose so mm1 of group 0 can
                        # start as soon as the first 256 KiB lands.
                        q = CHUNK_PAIRS // 4
                        for k in range(4):
                            nc.sync.dma_start_transpose(
                                out=rt[:, k * q:(k + 1) * q],
                                in_=rp[k * q:(k + 1) * q, :],
                            )
                    else:
                        nc.sync.dma_start_transpose(
                            out=rt[:],
                            in_=rp[c * CHUNK_PAIRS:(c + 1) * CHUNK_PAIRS, :],
                        )
                    e1w = e1_pool.tile([D, PW], F32, tag="e1")
                    if pending is not None:
                        e2w = e2_pool.tile([D, PW], F32, tag="e2")
                    else:
                        e2w = None
                    for h in range(H):
                        stage1_half(rt, e1w, h)
                        if pending is not None:
                            stage2_half(pending[0], e2w, h)
                    a1w = a1_pool.tile([D, PW], BF16, tag="a1")
                    nc.scalar.activation(a1w[:], e1w[:], f_ln, bias=1.0)
                    if pending is not None:
                        stage2_tail(e2w, pending[1])
                    pending = (a1w, c * I_PAIR)
                # flush last chunk tile: no stage1 work remains to overlap,
                # so double-buffer the mm2->Exp2 chain across BOTH psum pools
                # (f1's banks are free once its last Exp is done).
                e2w = e2_pool.tile([D, PW], F32, tag="e2")
                for h in range(H):
                    if h % 2 == 0:
                        stage2_half(pending[0], e2w, h)
                    else:
                        stage2_half(pending[0], e2w, h,
                                    pool=f1_pool, ptag="f1")
                stage2_tail(e2w, pending[1])

            if reps == 1:
                body()
            else:
                with tc.For_i(0, reps, 1):
                    body()

            nc.sync.dma_start(outT[:], out_sb[:])

    nc.compile()
    return nc


def _prepare_inputs(x, r, W1, b1, W2, b2):
    bf16 = ml_dtypes.bfloat16
    W1 = np.asarray(W1, np.float32)
    W2 = np.asarray(W2, np.float32)
    w1s = np.concatenate([W1, W1], axis=0).astype(bf16)          # [128, 128]
    w2b = W2.astype(bf16)                                        # [128, 128]
    b1c = np.asarray(b1, np.float32).reshape(D, 1)
    b2p = (np.asarray(b2, np.float32)
           - LOG2 * W2.sum(axis=0)).reshape(D, 1)

    in_maps = []
    for b in range(B):
        xbT = np.asarray(x[b], np.float32).T                     # [128 d, 256 j]
        in_maps.append({
            "rp": np.ascontiguousarray(
                np.asarray(r[b], np.float32).reshape(PAIRS, 2 * RBF)
            ).astype(bf16),
            "xte": np.ascontiguousarray(xbT[:, 0::2]).astype(bf16),
            "xto": np.ascontiguousarray(xbT[:, 1::2]).astype(bf16),
            "corr": (-LOG2 * xbT.sum(axis=1, dtype=np.float64)
                     ).astype(np.float32).reshape(D, 1),
            "w1s": w1s,
            "w2": w2b,
            "b1c": b1c,
            "b2p": b2p,
        })
    return in_maps


_NC_CACHE = None


def _get_nc():
    global _NC_CACHE
    if _NC_CACHE is None:
        _NC_CACHE = _build_program()
    return _NC_CACHE


def hw_time_ns(inputs, reps=33, n_meas=3):
    """Measure on-device per-iteration time by comparing wall time of a
    reps-times device loop against a single-iteration run."""
    import time as _time
    in_maps = _prepare_inputs(**inputs)

    def run_with(nc_prog):
        ts = []
        for _ in range(n_meas):
            t0 = _time.time()
            run_bass_kernel_spmd(nc_prog, in_maps, list(range(N_CORES)))
            ts.append(_time.time() - t0)
        return min(ts)

    nc1 = _build_program(reps=1)
    ncr = _build_program(reps=reps)
    w1 = run_with(nc1)
    wr = run_with(ncr)
    return (wr - w1) / (reps - 1) * 1e9


def kernel(x, r, W1, b1, W2, b2, _trace=False, _trace_kwargs=None):
    nc = _get_nc()
    in_maps = _prepare_inputs(x, r, W1, b1, W2, b2)
    res = run_bass_kernel_spmd(
        nc, in_maps, list(range(N_CORES)),
        trace=_trace, **(_trace_kwargs or {}),
    )
    out = np.stack([
        np.asarray(res.results[b]["outT"], np.float32).T for b in range(B)
    ])
    if _trace:
        return out, res
    return out



# revision 2
# speedup vs baseline: 20.4530x; 20.4530x over previous
"""CFConv (gnn message passing) Trainium2 kernel.

Math (per batch b):
    f1 = ssp(r @ W1 + b1)            ssp(x) = softplus(x) - log2
    f2 = ssp(f1 @ W2 + b2)
    out[i, d] = sum_j x[j, d] * f2[i, j, d]

Sharding: data-parallel over batch B=8 across the 8 cores (one batch each).

softplus is evaluated in ONE ACT pass + one fused DVE op per layer via the
variational (Legendre) form
    softplus(z) = z*sigma(z) + S(sigma(z)),
    S(p) = -p ln p - (1-p) ln(1-p)   (binary entropy),
which is first-order INSENSITIVE to errors in p = sigma(z) (d/dp vanishes at
p = sigma(z)), so a bf16 p and an approximate S are both safe.  S is fitted as
    S(p) ~= d + w*(a + c*z^2),  w = p*(1-p)
(the w*z^2 term captures the -w*ln w tails since z ~= -+ln w there).  The
constant d costs nothing on device: layer-1's d folds into layer-2's bias row
(d1*sum_d W2[d,:]) and layer-2's d folds into the output correction vector
(corr += d2*sum_j x[j,d]).  End-to-end sim error of the full bf16 pipeline
vs the f64 reference: ~1.1e-2 (gate 2e-2).

The fused DVE op (8 ALU stages, the hardware max):
    out = z*p + w*(C0 + C2*z^2)      in0=z (f32, PSUM), in1=p (bf16, SBUF)
registered at import into dve_ops.OPS; the per-NEFF DVE table mechanism ships
it to the device (no firmware change).

Layer biases reach PSUM via a K=2 ones matmul whose stationary holds the bias
split hi/lo across two bf16 rows (residual error ~4e-6), accumulated with the
data matmuls through start/stop PSUM groups.  This keeps all three custom-op
scalar slots free for fit constants.

Per-core pipeline (features on partitions, (i,j)-pairs on the free dim):
  1. r pairs DMA-transposed to SBUF [128, pairs] bf16 (even j in partitions
     0:64, odd in 64:128), chunked 4096 pairs.
  2. per 2048-col PSUM group: bias matmuls (K=2 ones) + mm1 (two K=64
     row-group matmuls) -> z1; ACT Sigmoid -> p1 (bf16); fused DVE op
     -> a1 (bf16).
  3. mm2 (K=128) + bias2 matmuls -> z2; Sigmoid -> p2; fused op -> f2 (bf16).
  4. prod = f2 * xT broadcast, 3D tensor_reduce over j, even+odd add,
     + corr.  Output [d, i] on device; host transposes.
Layer-2 work of chunk c-1 interleaves with layer-1 of chunk c (software
pipelining), PSUM split 4+4 banks between the two layers' groups.
"""

import numpy as np
import ml_dtypes

import concourse.bass as bass
import concourse.tile as tile
from concourse import bacc, mybir
from concourse.bass_utils import run_bass_kernel_spmd

LOG2 = float(np.log(2.0))

B, N, D, RBF = 8, 256, 128, 64
PAIRS = N * N // 2            # 32768 row-pairs per batch
CHUNK_PAIRS = 4096            # pairs per DMA-transpose chunk (1 MiB)
GROUP_PAIRS = 1024            # pairs per PSUM group (8 query nodes i)
SUB = 512                     # cols per matmul (one PSUM bank)
I_PER_GROUP = GROUP_PAIRS // (N // 2)   # 8
H = CHUNK_PAIRS // GROUP_PAIRS          # groups per chunk tile (4)
N_CORES = 8

BF16 = mybir.dt.bfloat16
F32 = mybir.dt.float32

# Entropy-term fit constants: S(p) ~= d + a*w + c*w*z^2, w = p(1-p).
# Minimax-fitted over the empirical preactivation ranges (layer1 z ~ +-7,
# layer2 z ~ +-4.5); the d's are folded host-side (see module docstring).
D1_C, A1_C, C1_C = -0.00505643, 2.80592749, 0.17756259
D2_C, A2_C, C2_C = -0.01382355, 2.83344796, 0.19336128

_SOFTPLUS_OP = None


def _register_softplus_op():
    """Register the fused softplus-finish DVE op (idempotent).

    out = in0*in1 + w*(s0 + imm2*in0^2),  w = in1*(1-in1)
    """
    global _SOFTPLUS_OP
    if _SOFTPLUS_OP is not None:
        return _SOFTPLUS_OP
    import concourse.dve_ops as dve_ops

    name = "SOFTPLUS_VAR_FIN"
    for op in dve_ops.OPS:
        if op.name == name:
            _SOFTPLUS_OP = op
            return op

    from concourse.dve_ops import DveOp
    from concourse.dve_spec import C0, C2, One, Spec, Src0, Src1, lower, sq
    from concourse.dve_uop import DveOpSpec

    w = Src1 * (One - Src1)
    body = w * (sq(Src0) * C2 + C0) + Src0 * Src1

    def ref(in0, in1, s0, s1, imm2):
        in0 = np.asarray(in0, np.float32)
        in1 = np.asarray(in1, np.float32)
        wv = in1 * (1.0 - in1)
        return wv * (in0 * in0 * imm2 + s0) + in0 * in1

    spec = Spec(body=body, reference=ref)
    row = dve_ops._CUSTOM_DVE_ROW_BASE + len(dve_ops.OPS)
    shas = {}
    for ver in ("v3", "v4"):
        uops = lower(spec, ver=ver)
        shas[ver] = DveOpSpec(
            name=name, opcode=row, uops=uops, rd1_en=True
        ).sha(ver)
    op = DveOp(name, spec, subdim=False, uops_sha=shas)
    dve_ops.OPS.append(op)
    dve_ops._SUB_OPCODE_FOR_NAME[name] = row
    dve_ops.CUSTOM_DVE_SPECS[name] = spec
    _SOFTPLUS_OP = op
    return op


def _build_program(reps: int = 1):
    sp_op = _register_softplus_op()

    nc = bacc.Bacc("TRN2", target_bir_lowering=False, debug=False,
                   num_devices=N_CORES)

    rp = nc.dram_tensor("rp", [PAIRS, 2 * RBF], BF16, kind="ExternalInput").ap()
    xte = nc.dram_tensor("xte", [D, N // 2], BF16, kind="ExternalInput").ap()
    xto = nc.dram_tensor("xto", [D, N // 2], BF16, kind="ExternalInput").ap()
    corr = nc.dram_tensor("corr", [D, 1], F32, kind="ExternalInput").ap()
    w1s = nc.dram_tensor("w1s", [2 * RBF, D], BF16, kind="ExternalInput").ap()
    w2 = nc.dram_tensor("w2", [D, D], BF16, kind="ExternalInput").ap()
    bias1 = nc.dram_tensor("bias1", [2, D], BF16, kind="ExternalInput").ap()
    bias2 = nc.dram_tensor("bias2", [2, D], BF16, kind="ExternalInput").ap()
    ones = nc.dram_tensor("ones", [2, SUB], BF16, kind="ExternalInput").ap()
    outT = nc.dram_tensor("outT", [D, N], F32, kind="ExternalOutput").ap()

    f_sig = mybir.ActivationFunctionType.Sigmoid
    mult = mybir.AluOpType.mult
    add = mybir.AluOpType.add

    with tile.TileContext(nc) as tc:
        with (
            tc.tile_pool(name="const", bufs=1) as const,
            tc.tile_pool(name="rt", bufs=3) as rt_pool,
            tc.tile_pool(name="p1", bufs=2) as p1_pool,
            tc.tile_pool(name="p2", bufs=2) as p2_pool,
            tc.tile_pool(name="a1", bufs=2) as a1_pool,
            tc.tile_pool(name="f2", bufs=2) as f2_pool,
            tc.tile_pool(name="prod", bufs=2) as prod_pool,
            tc.tile_pool(name="acc", bufs=2) as acc_pool,
            tc.tile_pool(name="osb", bufs=1) as out_pool,
            tc.tile_pool(name="z1", bufs=1, space="PSUM") as z1_pool,
            tc.tile_pool(name="z2", bufs=1, space="PSUM") as z2_pool,
        ):
            w1s_t = const.tile([2 * RBF, D], BF16, tag="w1s")
            w2_t = const.tile([D, D], BF16, tag="w2")
            xte_t = const.tile([D, N // 2], BF16, tag="xte")
            xto_t = const.tile([D, N // 2], BF16, tag="xto")
            bias1_t = const.tile([2, D], BF16, tag="bias1")
            bias2_t = const.tile([2, D], BF16, tag="bias2")
            ones_t = const.tile([2, SUB], BF16, tag="ones")
            corr_t = const.tile([D, 1], F32, tag="corr")
            nc.sync.dma_start(w1s_t[:], w1s[:])
            nc.sync.dma_start(w2_t[:], w2[:])
            nc.sync.dma_start(xte_t[:], xte[:])
            nc.sync.dma_start(xto_t[:], xto[:])
            nc.sync.dma_start(bias1_t[:], bias1[:])
            nc.sync.dma_start(bias2_t[:], bias2[:])
            nc.sync.dma_start(ones_t[:], ones[:])
            nc.sync.dma_start(corr_t[:], corr[:])

            out_sb = out_pool.tile([D, N], F32, tag="osb")

            # Tiny warmup activation right after the const loads: hoists the
            # ~2.7us ACT table load to t~0 where it overlaps the first DMA.
            warm = acc_pool.tile([D, 1], F32, tag="warm")
            nc.scalar.activation(warm[:], corr_t[:], f_sig, bias=0.0)

            jw = N // 2
            G2 = 2 * GROUP_PAIRS          # z-cols per group (2048)
            PW = H * G2                   # z-cols per chunk tile (8192)
            I_PAIR = H * I_PER_GROUP      # 32 query nodes per chunk tile
            xe4 = xte_t[:, None, None, :].broadcast_to([D, H, I_PER_GROUP, jw])
            xo4 = xto_t[:, None, None, :].broadcast_to([D, H, I_PER_GROUP, jw])

            def stage1_group(rt, p1w, a1w, h):
                """bias+mm1 -> sigmoid -> softplus-finish for one group."""
                g0 = h * GROUP_PAIRS
                z1 = z1_pool.tile([D, G2], F32, tag="z1")
                for s in range(G2 // SUB):
                    nc.tensor.matmul(
                        z1[:, s * SUB:(s + 1) * SUB],
                        bias1_t[:], ones_t[:],
                        start=True, stop=False, skip_group_check=True,
                    )
                for s in range(GROUP_PAIRS // SUB):
                    cs = g0 + s * SUB
                    nc.tensor.matmul(
                        z1[:, s * SUB:(s + 1) * SUB],
                        w1s_t[0:RBF, :],
                        rt[0:RBF, cs:cs + SUB],
                        start=False, stop=True, skip_group_check=True,
                    )
                for s in range(GROUP_PAIRS // SUB):
                    cs = g0 + s * SUB
                    nc.tensor.matmul(
                        z1[:, GROUP_PAIRS + s * SUB:
                            GROUP_PAIRS + (s + 1) * SUB],
                        w1s_t[RBF:2 * RBF, :],
                        rt[RBF:2 * RBF, cs:cs + SUB],
                        start=False, stop=True, skip_group_check=True,
                    )
                sl = slice(h * G2, (h + 1) * G2)
                nc.scalar.activation(p1w[:, sl], z1[:], f_sig, bias=0.0)
                nc.vector._custom_dve(
                    sp_op, out=a1w[:, sl], in0=z1[:], in1=p1w[:, sl],
                    s0=A1_C, imm2=C1_C,
                )

            def stage2_group(a1w, p2w, f2w, h, pool=None, ptag="z2"):
                """bias+mm2 -> sigmoid -> softplus-finish for one group of the
                previous chunk."""
                z2 = (pool or z2_pool).tile([D, G2], F32, tag=ptag)
                for s in range(G2 // SUB):
                    nc.tensor.matmul(
                        z2[:, s * SUB:(s + 1) * SUB],
                        bias2_t[:], ones_t[:],
                        start=True, stop=False, skip_group_check=True,
                    )
                for s in range(G2 // SUB):
                    nc.tensor.matmul(
                        z2[:, s * SUB:(s + 1) * SUB],
                        w2_t[:],
                        a1w[:, h * G2 + s * SUB:h * G2 + (s + 1) * SUB],
                        start=False, stop=True, skip_group_check=True,
                    )
                sl = slice(h * G2, (h + 1) * G2)
                nc.scalar.activation(p2w[:, sl], z2[:], f_sig, bias=0.0)
                nc.vector._custom_dve(
                    sp_op, out=f2w[:, sl], in0=z2[:], in1=p2w[:, sl],
                    s0=A2_C, imm2=C2_C,
                )

            def chunk_tail(f2w, i0):
                """weighted j-reduction of one chunk's f2."""
                # prod = f2 * xT (x broadcast over h and i); reduce innermost
                # j; out[:, i] = even + odd + corr.
                # f2w cols = [h: [even 1024 | odd 1024]] * H
                prod = prod_pool.tile([D, PW], BF16, tag="prod")
                p4 = prod[:].rearrange(
                    "p (h par k j) -> p h par k j", h=H, par=2, j=jw)
                a4 = f2w[:].rearrange(
                    "p (h par k j) -> p h par k j", h=H, par=2, j=jw)
                nc.vector.tensor_tensor(
                    p4[:, :, 0, :, :], a4[:, :, 0, :, :], xe4, mult)
                nc.vector.tensor_tensor(
                    p4[:, :, 1, :, :], a4[:, :, 1, :, :], xo4, mult)
                # sums index m = h*16 + par*8 + k
                sums = acc_pool.tile([D, 2 * I_PAIR], F32, tag="sums")
                nc.vector.tensor_reduce(
                    sums[:],
                    prod[:].rearrange("p (m j) -> p m j", j=jw),
                    axis=mybir.AxisListType.X,
                    op=add,
                )
                tmp = acc_pool.tile([D, I_PAIR], F32, tag="tmp")
                s4 = sums[:].rearrange(
                    "p (h par k) -> p h par k", h=H, par=2)
                nc.vector.tensor_add(
                    tmp[:].rearrange("p (h k) -> p h k", h=H),
                    s4[:, :, 0, :], s4[:, :, 1, :])
                nc.vector.tensor_scalar_add(
                    out_sb[:, i0:i0 + I_PAIR], tmp[:], corr_t[:])

            # Software-pipelined emission interleaving groups of chunk c's
            # layer 1 with groups of chunk c-1's layer 2.
            def body():
                pending = None  # (a1w, i0) of the previous chunk
                for c in range(PAIRS // CHUNK_PAIRS):
                    rt = rt_pool.tile([2 * RBF, CHUNK_PAIRS], BF16, tag="rt")
                    if c == 0:
                        # Quarter the first transpose so mm1 of group 0 can
                        # start as soon as the first 256 KiB lands.
                        qq = CHUNK_PAIRS // 4
                        for k in range(4):
                            nc.sync.dma_start_transpose(
                                out=rt[:, k * qq:(k + 1) * qq],
                                in_=rp[k * qq:(k + 1) * qq, :],
                            )
                    else:
                        nc.sync.dma_start_transpose(
                            out=rt[:],
                            in_=rp[c * CHUNK_PAIRS:(c + 1) * CHUNK_PAIRS, :],
                        )
                    p1w = p1_pool.tile([D, PW], BF16, tag="p1")
                    a1w = a1_pool.tile([D, PW], BF16, tag="a1")
                    if pending is not None:
                        p2w = p2_pool.tile([D, PW], BF16, tag="p2")
                        f2w = f2_pool.tile([D, PW], BF16, tag="f2")
                    for h in range(H):
                        stage1_group(rt, p1w, a1w, h)
                        if pending is not None:
                            stage2_group(pending[0], p2w, f2w, h)
                    if pending is not None:
                        chunk_tail(f2w, pending[1])
                    pending = (a1w, c * I_PAIR)
                # flush the last chunk: no stage1 work remains to overlap, so
                # double-buffer layer 2 across BOTH psum pools.
                p2w = p2_pool.tile([D, PW], BF16, tag="p2")
                f2w = f2_pool.tile([D, PW], BF16, tag="f2")
                for h in range(H):
                    if h % 2 == 0:
                        stage2_group(pending[0], p2w, f2w, h)
                    else:
                        stage2_group(pending[0], p2w, f2w, h,
                                     pool=z1_pool, ptag="z1")
                chunk_tail(f2w, pending[1])

            if reps == 1:
                body()
            else:
                with tc.For_i(0, reps, 1):
                    body()

            nc.sync.dma_start(outT[:], out_sb[:])

    nc.compile()
    return nc


def _prepare_inputs(x, r, W1, b1, W2, b2):
    bf16 = ml_dtypes.bfloat16
    W1 = np.asarray(W1, np.float32)
    W2 = np.asarray(W2, np.float32)
    w1s = np.concatenate([W1, W1], axis=0).astype(bf16)          # [128, 128]
    w2b = W2.astype(bf16)                                        # [128, 128]
    b1f = np.asarray(b1, np.float64)
    # layer-2 bias with the -log2 shift of layer 1's ssp and layer-1's fit
    # constant d1 folded through W2's column sums.
    b2n = (np.asarray(b2, np.float64)
           + (D1_C - LOG2) * W2.astype(np.float64).sum(axis=0))

    def hilo(v):
        hi = v.astype(np.float32).astype(bf16)
        lo = (v - hi.astype(np.float64)).astype(np.float32).astype(bf16)
        return np.stack([hi, lo])                                # [2, D]

    bias1 = hilo(b1f)
    bias2 = hilo(b2n)
    ones = np.ones((2, SUB), dtype=bf16)

    in_maps = []
    for b in range(B):
        xbT = np.asarray(x[b], np.float32).T                     # [128 d, 256 j]
        in_maps.append({
            "rp": np.ascontiguousarray(
                np.asarray(r[b], np.float32).reshape(PAIRS, 2 * RBF)
            ).astype(bf16),
            "xte": np.ascontiguousarray(xbT[:, 0::2]).astype(bf16),
            "xto": np.ascontiguousarray(xbT[:, 1::2]).astype(bf16),
            # layer-2's -log2 shift and fit constant d2, scaled by sum_j x.
            "corr": ((D2_C - LOG2) * xbT.sum(axis=1, dtype=np.float64)
                     ).astype(np.float32).reshape(D, 1),
            "w1s": w1s,
            "w2": w2b,
            "bias1": bias1,
            "bias2": bias2,
            "ones": ones,
        })
    return in_maps


_NC_CACHE = None


def _get_nc():
    global _NC_CACHE
    if _NC_CACHE is None:
        _NC_CACHE = _build_program()
    return _NC_CACHE


def hw_time_ns(inputs, reps=2049, n_meas=4):
    """Measure on-device per-iteration time by comparing wall time of a
    reps-times device loop against a single-iteration run."""
    import time as _time
    in_maps = _prepare_inputs(**inputs)

    def run_with(nc_prog):
        ts = []
        for _ in range(n_meas):
            t0 = _time.time()
            run_bass_kernel_spmd(nc_prog, in_maps, list(range(N_CORES)))
            ts.append(_time.time() - t0)
        return min(ts)

    nc1 = _build_program(reps=1)
    ncr = _build_program(reps=reps)
    w1 = run_with(nc1)
    wr = run_with(ncr)
    return (wr - w1) / (reps - 1) * 1e9


def kernel(x, r, W1, b1, W2, b2, _trace=False, _trace_kwargs=None):
    nc = _get_nc()
    in_maps = _prepare_inputs(x, r, W1, b1, W2, b2)
    res = run_bass_kernel_spmd(
        nc, in_maps, list(range(N_CORES)),
        trace=_trace, **(_trace_kwargs or {}),
    )
    out = np.stack([
        np.asarray(res.results[b]["outT"], np.float32).T for b in range(B)
    ])
    if _trace:
        return out, res
    return out


# revision 19
# speedup vs baseline: 25.6690x; 1.2550x over previous
"""CFConv (gnn message passing) Trainium2 kernel.

Math (per batch b):
    f1 = ssp(r @ W1 + b1)            ssp(x) = softplus(x) - log2
    f2 = ssp(f1 @ W2 + b2)
    out[i, d] = sum_j x[j, d] * f2[i, j, d]

Sharding: data-parallel over batch B=8 across the 8 cores (one batch each).

softplus is evaluated in ONE ACT pass + one fused DVE op per layer via the
variational (Legendre) form
    softplus(z) = z*sigma(z) + S(sigma(z)),
    S(p) = -p ln p - (1-p) ln(1-p)   (binary entropy),
which is first-order INSENSITIVE to errors in p = sigma(z) (d/dp vanishes at
p = sigma(z)), so a bf16 p and an approximate S are both safe.  S is fitted as
    S(p) ~= d + w*(a + c*z^2),  w = p*(1-p)
(the w*z^2 term captures the -w*ln w tails since z ~= -+ln w there).  The
constant d costs nothing on device: layer-1's d folds into layer-2's bias row
(d1*sum_d W2[d,:]) and layer-2's d folds into the output correction vector
(corr += d2*sum_j x[j,d]).

The fused DVE op (8 ALU stages, the hardware max):
    out = z*p + w*(C0 + C2*z^2)      in0=z (f32, PSUM), in1=p (bf16, SBUF)

Engine budget per core (~65k cols/lane per layer): ACT 2 sigmoid passes
~120us; DVE 2 fused passes (1 elem/lane/cyc @0.96GHz) + the f2*x products
~175us; PE matmuls ~90-160us; the j-reduction runs as a 7-level f32
tensor_add tree on the otherwise-idle GPSIMD (Pool) engine ~130us.  DVE is
the roofline.

Bias handling (keeps all three custom-op scalar slots free for fit consts):
 - layer 1: b1 (~0.03) enters sigma via the ACT affine; its silu-term effect
   b1*p is folded host-side as 0.5*b1^T W2 into the layer-2 bias (residual
   ~(p-0.5)*b1 is negligible).  mm1 output stays raw.
 - layer 2: bias reaches PSUM exactly via a K=2 ones matmul whose stationary
   holds the bias split hi/lo across two bf16 rows (error ~4e-6).
End-to-end sim error of the full bf16 pipeline vs the f64 reference:
~1.2e-2 (gate 2e-2).

Per-core pipeline (features on partitions, (i,j)-pairs on the free dim):
  r pairs DMA-transposed to SBUF [128, pairs] bf16 (even j in partitions
  0:64, odd in 64:128), chunked 4096 pairs.  Layer 1 works in 2048-col PSUM
  groups (4 banks, single-buffered); layer 2 in 1024-col half-groups
  (2 banks, double-buffered) so its longer PE->ACT->DVE chain pipelines.
  Layer-2 work of chunk c-1 interleaves with layer-1 of chunk c.
"""

import numpy as np
import ml_dtypes

import concourse.bass as bass
import concourse.tile as tile
from concourse import bacc, mybir
from concourse.bass_utils import run_bass_kernel_spmd

LOG2 = float(np.log(2.0))

B, N, D, RBF = 8, 256, 128, 64
PAIRS = N * N // 2            # 32768 row-pairs per batch
CHUNK_PAIRS = 4096            # pairs per DMA-transpose chunk (1 MiB)
GROUP_PAIRS = 1024            # pairs per layer-1 PSUM group
SUB = 512                     # cols per matmul (one PSUM bank)
HG = 1024                     # cols per layer-2 PSUM half-group
I_PER_GROUP = GROUP_PAIRS // (N // 2)   # 8
H = CHUNK_PAIRS // GROUP_PAIRS          # groups per chunk tile (4)
N_CORES = 8

BF16 = mybir.dt.bfloat16
F32 = mybir.dt.float32

# Entropy-term fit constants: S(p) ~= d + a*w + c*w*z^2, w = p(1-p).
# Minimax-fitted over the empirical preactivation ranges (layer1 z ~ +-7,
# layer2 z ~ +-4.5); the d's are folded host-side (see module docstring).
D1_C, A1_C, C1_C = -0.00505643, 2.80592749, 0.17756259
D2_C, A2_C, C2_C = -0.01382355, 2.83344796, 0.19336128

_SOFTPLUS_OP = None


def _register_softplus_op():
    """Register the fused softplus-finish DVE op (idempotent).

    out = in0*in1 + w*(s0 + imm2*in0^2),  w = in1*(1-in1)
    """
    global _SOFTPLUS_OP
    if _SOFTPLUS_OP is not None:
        return _SOFTPLUS_OP
    import concourse.dve_ops as dve_ops

    name = "SOFTPLUS_VAR_FIN"
    for op in dve_ops.OPS:
        if op.name == name:
            _SOFTPLUS_OP = op
            return op

    from concourse.dve_ops import DveOp
    from concourse.dve_spec import C0, C2, One, Spec, Src0, Src1, lower, sq
    from concourse.dve_uop import DveOpSpec

    w = Src1 * (One - Src1)
    body = w * (sq(Src0) * C2 + C0) + Src0 * Src1

    def ref(in0, in1, s0, s1, imm2):
        in0 = np.asarray(in0, np.float32)
        in1 = np.asarray(in1, np.float32)
        wv = in1 * (1.0 - in1)
        return wv * (in0 * in0 * imm2 + s0) + in0 * in1

    spec = Spec(body=body, reference=ref)
    row = dve_ops._CUSTOM_DVE_ROW_BASE + len(dve_ops.OPS)
    shas = {}
    for ver in ("v3", "v4"):
        uops = lower(spec, ver=ver)
        shas[ver] = DveOpSpec(
            name=name, opcode=row, uops=uops, rd1_en=True
        ).sha(ver)
    op = DveOp(name, spec, subdim=False, uops_sha=shas)
    dve_ops.OPS.append(op)
    dve_ops._SUB_OPCODE_FOR_NAME[name] = row
    dve_ops.CUSTOM_DVE_SPECS[name] = spec
    _SOFTPLUS_OP = op
    return op


def _build_program(reps: int = 1, unroll: int = 1):
    sp_op = _register_softplus_op()

    nc = bacc.Bacc("TRN2", target_bir_lowering=False, debug=False,
                   num_devices=N_CORES)

    rp = nc.dram_tensor("rp", [PAIRS, 2 * RBF], BF16, kind="ExternalInput").ap()
    xte = nc.dram_tensor("xte", [D, N // 2], BF16, kind="ExternalInput").ap()
    xto = nc.dram_tensor("xto", [D, N // 2], BF16, kind="ExternalInput").ap()
    corr = nc.dram_tensor("corr", [D, 1], F32, kind="ExternalInput").ap()
    w1s = nc.dram_tensor("w1s", [2 * RBF, D], BF16, kind="ExternalInput").ap()
    w2 = nc.dram_tensor("w2", [D, D], BF16, kind="ExternalInput").ap()
    b1c = nc.dram_tensor("b1c", [D, 1], F32, kind="ExternalInput").ap()
    bias2 = nc.dram_tensor("bias2", [2, D], BF16, kind="ExternalInput").ap()
    ones = nc.dram_tensor("ones", [2, SUB], BF16, kind="ExternalInput").ap()
    outT = nc.dram_tensor("outT", [D, N], F32, kind="ExternalOutput").ap()

    f_sig = mybir.ActivationFunctionType.Sigmoid
    mult = mybir.AluOpType.mult

    with tile.TileContext(nc) as tc:
        with (
            tc.tile_pool(name="const", bufs=1) as const,
            tc.tile_pool(name="rt", bufs=3) as rt_pool,
            tc.tile_pool(name="p1", bufs=2) as p1_pool,
            tc.tile_pool(name="p2", bufs=2) as p2_pool,
            tc.tile_pool(name="a1", bufs=2) as a1_pool,
            tc.tile_pool(name="f2", bufs=2) as f2_pool,
            tc.tile_pool(name="prod", bufs=2) as prod_pool,
            tc.tile_pool(name="tree", bufs=1) as tree_pool,
            tc.tile_pool(name="acc", bufs=2) as acc_pool,
            tc.tile_pool(name="osb", bufs=1) as out_pool,
            tc.tile_pool(name="z1", bufs=2, space="PSUM") as z1_pool,
            tc.tile_pool(name="z2", bufs=2, space="PSUM") as z2_pool,
        ):
            w1s_t = const.tile([2 * RBF, D], BF16, tag="w1s")
            w2_t = const.tile([D, D], BF16, tag="w2")
            xte_t = const.tile([D, N // 2], BF16, tag="xte")
            xto_t = const.tile([D, N // 2], BF16, tag="xto")
            b1_t = const.tile([D, 1], F32, tag="b1")
            bias2_t = const.tile([2, D], BF16, tag="bias2")
            ones_t = const.tile([2, SUB], BF16, tag="ones")
            corr_t = const.tile([D, 1], F32, tag="corr")
            nc.sync.dma_start(w1s_t[:], w1s[:])
            nc.sync.dma_start(b1_t[:], b1c[:])
            nc.sync.dma_start(w2_t[:], w2[:])
            nc.sync.dma_start(xte_t[:], xte[:])
            nc.sync.dma_start(xto_t[:], xto[:])
            nc.sync.dma_start(bias2_t[:], bias2[:])
            nc.sync.dma_start(ones_t[:], ones[:])
            nc.sync.dma_start(corr_t[:], corr[:])

            out_sb = out_pool.tile([D, N], F32, tag="osb")

            # Tiny warmup activation right after the const loads: hoists the
            # ~2.7us ACT table load to t~0 where it overlaps the first DMA.
            warm = acc_pool.tile([D, 1], F32, tag="warm")
            nc.scalar.activation(warm[:], b1_t[:], f_sig, bias=0.0)

            jw = N // 2
            G2 = 2 * GROUP_PAIRS          # z-cols per layer-1 group (2048)
            PW = H * G2                   # z-cols per chunk tile (8192)
            I_PAIR = H * I_PER_GROUP      # 32 query nodes per chunk tile
            M = PW // jw                  # reduce segments per chunk (64)

            def stage1_half(rt, a1w, hh):
                """mm1 -> sigmoid(+b1) -> softplus-finish for one 1024-col
                half-group (one j-parity of one group) of the current chunk.

                Half-groups + a double-buffered 2-bank PSUM pool give the
                mm1 -> sigma -> finish chain two steps of slack, so the
                in-order engine queues pipeline instead of serializing."""
                h, par = hh // 2, hh % 2
                g0 = h * GROUP_PAIRS
                r0, r1 = par * RBF, (par + 1) * RBF
                z1 = z1_pool.tile([D, HG], F32, tag="z1")
                for s in range(HG // SUB):
                    cs = g0 + s * SUB
                    nc.tensor.matmul(
                        z1[:, s * SUB:(s + 1) * SUB],
                        w1s_t[r0:r1, :],
                        rt[r0:r1, cs:cs + SUB],
                    )
                p1 = p1_pool.tile([D, HG], BF16, tag="p1")
                nc.scalar.activation(p1[:], z1[:], f_sig, bias=b1_t[:])
                nc.vector._custom_dve(
                    sp_op, out=a1w[:, hh * HG:(hh + 1) * HG], in0=z1[:],
                    in1=p1[:], s0=A1_C, imm2=C1_C,
                )

            def stage2_half(a1w, f2w, prod, hh):
                """bias+mm2 -> sigmoid -> softplus-finish -> *x for one
                1024-col half-group of the previous chunk.

                The f2*x product is emitted per half-group so the small TT
                never head-of-line blocks a PSUM-freeing custom op in the
                in-order DVE queue."""
                c0 = hh * HG
                z2 = z2_pool.tile([D, HG], F32, tag="z2")
                for s in range(HG // SUB):
                    nc.tensor.matmul(
                        z2[:, s * SUB:(s + 1) * SUB],
                        bias2_t[:], ones_t[:],
                        start=True, stop=False, skip_group_check=True,
                    )
                for s in range(HG // SUB):
                    nc.tensor.matmul(
                        z2[:, s * SUB:(s + 1) * SUB],
                        w2_t[:],
                        a1w[:, c0 + s * SUB:c0 + (s + 1) * SUB],
                        start=False, stop=True, skip_group_check=True,
                    )
                p2 = p2_pool.tile([D, HG], BF16, tag="p2")
                nc.scalar.activation(p2[:], z2[:], f_sig, bias=0.0)
                nc.vector._custom_dve(
                    sp_op, out=f2w[:, c0:c0 + HG], in0=z2[:],
                    in1=p2[:], s0=A2_C, imm2=C2_C,
                )
                xb = (xte_t if hh % 2 == 0 else xto_t)[:, None, :]
                # 2 of 8 slices per chunk go to Pool (2.0us there vs 0.59 on
                # DVE) to shave the DVE roofline; Pool has ~45us of headroom.
                eng = nc.gpsimd if hh in (3, 7) else nc.vector
                eng.tensor_tensor(
                    prod[:, c0:c0 + HG].rearrange("p (k j) -> p k j", j=jw),
                    f2w[:, c0:c0 + HG].rearrange("p (k j) -> p k j", j=jw),
                    xb.broadcast_to([D, I_PER_GROUP, jw]),
                    mult,
                )

            def chunk_tail(prod, i0):
                """j-reduction of one chunk's f2*x products: a 7-level f32
                tensor_add tree on the Pool engine, which is otherwise idle.
                """
                m3 = prod[:].rearrange("p (m j) -> p m j", j=jw)
                t = m3
                for lvl in range(7):
                    half = jw >> (lvl + 1)
                    nxt = tree_pool.tile([D, M, half], F32, tag=f"t{lvl}")
                    nc.gpsimd.tensor_add(
                        nxt[:], t[:, :, 0:half], t[:, :, half:2 * half])
                    t = nxt
                # t is [D, M, 1]; segments m = h*16 + par*8 + k
                s4 = t[:].rearrange(
                    "p (h par k) o -> p h par (k o)", h=H, par=2)
                tmp = acc_pool.tile([D, I_PAIR], F32, tag="tmp")
                nc.gpsimd.tensor_add(
                    tmp[:].rearrange("p (h k) -> p h k", h=H),
                    s4[:, :, 0, :], s4[:, :, 1, :])
                nc.gpsimd.tensor_scalar_add(
                    out_sb[:, i0:i0 + I_PAIR], tmp[:], corr_t[:])

            def group_tail(prod, g, i0):
                """per-group (16-segment) variant of chunk_tail, used in the
                final flush so the tree overlaps the remaining layer-2 work
                instead of trailing it."""
                m3 = prod[:].rearrange("p (m j) -> p m j", j=jw)
                t = m3[:, 16 * g:16 * (g + 1), :]
                for lvl in range(7):
                    half = jw >> (lvl + 1)
                    nxt = tree_pool.tile([D, 16, half], F32, tag=f"g{lvl}")
                    nc.gpsimd.tensor_add(
                        nxt[:], t[:, :, 0:half], t[:, :, half:2 * half])
                    t = nxt
                s4 = t[:].rearrange("p (par k) o -> p par (k o)", par=2)
                tmp = acc_pool.tile([D, I_PER_GROUP], F32, tag="tmpg")
                nc.gpsimd.tensor_add(tmp[:], s4[:, 0, :], s4[:, 1, :])
                nc.gpsimd.tensor_scalar_add(
                    out_sb[:, i0 + 8 * g:i0 + 8 * (g + 1)], tmp[:], corr_t[:])

            # Software-pipelined emission interleaving groups of chunk c's
            # layer 1 with half-groups of chunk c-1's layer 2.
            def body():
                pending = None  # (a1w, i0) of the previous chunk
                for c in range(PAIRS // CHUNK_PAIRS):
                    rt = rt_pool.tile([2 * RBF, CHUNK_PAIRS], BF16, tag="rt")
                    if c == 0:
                        # Slice the first transpose 8 ways so mm1 of the
                        # first half-group starts as soon as 128 KiB lands.
                        qq = CHUNK_PAIRS // 8
                        for k in range(8):
                            nc.sync.dma_start_transpose(
                                out=rt[:, k * qq:(k + 1) * qq],
                                in_=rp[k * qq:(k + 1) * qq, :],
                            )
                    else:
                        nc.sync.dma_start_transpose(
                            out=rt[:],
                            in_=rp[c * CHUNK_PAIRS:(c + 1) * CHUNK_PAIRS, :],
                        )
                    a1w = a1_pool.tile([D, PW], BF16, tag="a1")
                    if pending is not None:
                        f2w = f2_pool.tile([D, PW], BF16, tag="f2")
                        prod = prod_pool.tile([D, PW], BF16, tag="prod")
                    for h in range(H):
                        stage1_half(rt, a1w, 2 * h)
                        if pending is not None:
                            stage2_half(pending[0], f2w, prod, 2 * h)
                        stage1_half(rt, a1w, 2 * h + 1)
                        if pending is not None:
                            stage2_half(pending[0], f2w, prod, 2 * h + 1)
                    if pending is not None:
                        chunk_tail(prod, pending[1])
                    pending = (a1w, c * I_PAIR)
                # flush the last chunk's layer 2 (z2 pool double-buffers);
                # per-group tails so the Pool tree overlaps the layer-2 work.
                f2w = f2_pool.tile([D, PW], BF16, tag="f2")
                prod = prod_pool.tile([D, PW], BF16, tag="prod")
                for hh in range(2 * H):
                    stage2_half(pending[0], f2w, prod, hh)
                    if hh % 2 == 1:
                        group_tail(prod, hh // 2, pending[1])

            if unroll > 1:
                for _ in range(unroll):
                    body()
            elif reps == 1:
                body()
            else:
                with tc.For_i(0, reps, 1):
                    body()

            nc.sync.dma_start(outT[:], out_sb[:])

    nc.compile()
    return nc


def _prepare_inputs(x, r, W1, b1, W2, b2):
    bf16 = ml_dtypes.bfloat16
    W1 = np.asarray(W1, np.float32)
    W2 = np.asarray(W2, np.float32)
    W2d = W2.astype(np.float64)
    b1d = np.asarray(b1, np.float64)
    w1s = np.concatenate([W1, W1], axis=0).astype(bf16)          # [128, 128]
    w2b = W2.astype(bf16)                                        # [128, 128]
    # layer-2 bias with: the -log2 shift of layer 1's ssp, layer-1's fit
    # constant d1 (both through W2's column sums), and the mean effect of
    # b1's silu term (0.5 * b1^T W2; bias1 has no PSUM matmul of its own).
    b2n = (np.asarray(b2, np.float64)
           + (D1_C - LOG2) * W2d.sum(axis=0)
           + 0.5 * (b1d @ W2d))

    hi = b2n.astype(np.float32).astype(bf16)
    lo = (b2n - hi.astype(np.float64)).astype(np.float32).astype(bf16)
    bias2 = np.stack([hi, lo])                                   # [2, D]
    ones = np.ones((2, SUB), dtype=bf16)
    b1c = b1d.astype(np.float32).reshape(D, 1)

    in_maps = []
    for b in range(B):
        xbT = np.asarray(x[b], np.float32).T                     # [128 d, 256 j]
        in_maps.append({
            "rp": np.ascontiguousarray(
                np.asarray(r[b], np.float32).reshape(PAIRS, 2 * RBF)
            ).astype(bf16),
            "xte": np.ascontiguousarray(xbT[:, 0::2]).astype(bf16),
            "xto": np.ascontiguousarray(xbT[:, 1::2]).astype(bf16),
            # layer-2's -log2 shift and fit constant d2, scaled by sum_j x.
            "corr": ((D2_C - LOG2) * xbT.sum(axis=1, dtype=np.float64)
                     ).astype(np.float32).reshape(D, 1),
            "w1s": w1s,
            "w2": w2b,
            "b1c": b1c,
            "bias2": bias2,
            "ones": ones,
        })
    return in_maps


_NC_CACHE = None


def _get_nc():
    global _NC_CACHE
    if _NC_CACHE is None:
        _NC_CACHE = _build_program()
    return _NC_CACHE


def hw_time_ns(inputs, reps=2049, n_meas=4):
    """Measure on-device per-iteration time by comparing wall time of a
    reps-times device loop against a single-iteration run."""
    import time as _time
    in_maps = _prepare_inputs(**inputs)

    def run_with(nc_prog):
        ts = []
        for _ in range(n_meas):
            t0 = _time.time()
            run_bass_kernel_spmd(nc_prog, in_maps, list(range(N_CORES)))
            ts.append(_time.time() - t0)
        return min(ts)

    nc1 = _build_program(reps=1)
    ncr = _build_program(reps=reps)
    w1 = run_with(nc1)
    wr = run_with(ncr)
    return (wr - w1) / (reps - 1) * 1e9


def kernel(x, r, W1, b1, W2, b2, _trace=False, _trace_kwargs=None):
    nc = _get_nc()
    in_maps = _prepare_inputs(x, r, W1, b1, W2, b2)
    res = run_bass_kernel_spmd(
        nc, in_maps, list(range(N_CORES)),
        trace=_trace, **(_trace_kwargs or {}),
    )
    out = np.stack([
        np.asarray(res.results[b]["outT"], np.float32).T for b in range(B)
    ])
    if _trace:
        return out, res
    return out
